# revision 12
# baseline (speedup 1.0000x reference)
"""Trainium2 Bass kernel for nn_Attention_65601330479503.

Cluster-pattern-biased multi-head attention:
  qkv = x @ Wqkv; attn = softmax(q k^T/sqrt(hd) + cp_bias); out = (attn v) @ Wproj
  cp_bias = cp_scale * theta_i theta_j * tanh(zB_i . z_j),  z = softmax(q Wgroup)

Sharding: 8 cores, core c -> batch b = c//2, head-group hg = (c%2)*8 (8 of 16
heads).  Each core computes its 8 heads' attention fully on-device; the host
adds the two per-batch partial output projections and transposes the per-core
cp_bias^T slabs into the full [B, H, N, N] output.

On-device orientation is TRANSPOSED ([key m on partitions, query n on free])
so that softmax sums come free via an appended ones-column in v (row 64 of the
AV accumulator) and attn@v needs no P-matrix transposes.  cp_bias is emitted
as [m, n] tiles and un-transposed on the host during the gather.
"""

import sys

import numpy as np

sys.path.insert(0, "/opt/trn_rl_repo")

N = 1024          # sequence length
D = 1024          # model dim
HD = 64           # head dim
NHL = 8           # heads per core (local)
KC = 8            # clusters
NT = N // 128     # 8 n-tiles
SCALE = HD ** -0.5

_graph_cache = {}


def _build(with_bias: bool):
    from contextlib import ExitStack

    import concourse.bass as bass
    import concourse.tile as tile
    from concourse import bacc, mybir
    from concourse.masks import make_identity

    f32 = mybir.dt.float32
    f32r = mybir.dt.float32r
    bf16 = mybir.dt.bfloat16
    AF = mybir.ActivationFunctionType
    ALU = mybir.AluOpType

    nc = bacc.Bacc("TRN2", target_bir_lowering=False, debug=False, num_devices=8)

    xb = nc.dram_tensor("xb", [N, D], f32, kind="ExternalInput").ap()
    wq = nc.dram_tensor("wq", [D, NHL * HD], f32r, kind="ExternalInput").ap()
    wk = nc.dram_tensor("wk", [D, NHL * HD], f32r, kind="ExternalInput").ap()
    wv = nc.dram_tensor("wv", [D, NHL * HD], f32r, kind="ExternalInput").ap()
    wp = nc.dram_tensor("wp", [NHL * HD, D], f32r, kind="ExternalInput").ap()
    o_part = nc.dram_tensor("o_part", [N, D], f32, kind="ExternalOutput").ap()
    if with_bias:
        wg = nc.dram_tensor("wg", [HD, KC], f32, kind="ExternalInput").ap()
        ebg = nc.dram_tensor("ebg", [1, KC], f32, kind="ExternalInput").ap()
        bgr = nc.dram_tensor("bgr", [1, KC], f32, kind="ExternalInput").ap()
        wt = nc.dram_tensor("wt", [HD, 1], f32, kind="ExternalInput").ap()
        bt = nc.dram_tensor("bt", [1, 1], f32, kind="ExternalInput").ap()
        aff = nc.dram_tensor("aff", [KC, NHL * KC], f32, kind="ExternalInput").ap()
        cps = nc.dram_tensor("cps", [1, 1], f32, kind="ExternalInput").ap()
        cpbT = nc.dram_tensor("cpbT", [NHL, N, N], f32, kind="ExternalOutput").ap()
        sz = nc.dram_tensor("sz", [NHL, 128, NT], f32, kind="ExternalOutput").ap()
        zg = nc.dram_tensor("zg", [NHL, 128, NT], f32, kind="ExternalOutput").ap()

    with tile.TileContext(nc) as tc, ExitStack() as ctx:
        const = ctx.enter_context(tc.tile_pool(name="const", bufs=1))
        persist = ctx.enter_context(tc.tile_pool(name="persist", bufs=1))
        xrow = ctx.enter_context(tc.tile_pool(name="xrow", bufs=2))
        wblk = ctx.enter_context(tc.tile_pool(name="wblk", bufs=8))
        ptile = ctx.enter_context(tc.tile_pool(name="ptile", bufs=2))
        work = ctx.enter_context(tc.tile_pool(name="work", bufs=3))
        cpbp = ctx.enter_context(tc.tile_pool(name="cpbp", bufs=3))
        bcast = ctx.enter_context(tc.tile_pool(name="bcast", bufs=2))
        smalls = ctx.enter_context(tc.tile_pool(name="smalls", bufs=2))
        tail = ctx.enter_context(tc.tile_pool(name="tail", bufs=2))
        opool = ctx.enter_context(tc.tile_pool(name="opool", bufs=2))
        pa = ctx.enter_context(tc.tile_pool(name="pa", bufs=4, space="PSUM"))
        pav = ctx.enter_context(tc.tile_pool(name="pav", bufs=2, space="PSUM"))
        psm = ctx.enter_context(tc.tile_pool(name="psm", bufs=2, space="PSUM"))

        identity = const.tile([128, 128], f32)
        make_identity(nc, identity)

        # ---- constants ----
        if with_bias:
            # wg/wt duplicated into both partition halves so base-64 head
            # slices have a matching-base operand
            wg_sb = const.tile([128, KC], f32, tag="wg")
            nc.sync.dma_start(out=wg_sb[0:HD, :], in_=wg)
            nc.sync.dma_start(out=wg_sb[HD:2 * HD, :], in_=wg)
            wt_sb = const.tile([128, 1], f32, tag="wt")
            nc.sync.dma_start(out=wt_sb[0:HD, :], in_=wt)
            nc.sync.dma_start(out=wt_sb[HD:2 * HD, :], in_=wt)
            aff_sb = const.tile([KC, NHL * KC], bf16, tag="aff")
            nc.gpsimd.dma_start(out=aff_sb, in_=aff)
            ebg_row = const.tile([1, KC], f32, tag="ebgr")
            nc.sync.dma_start(out=ebg_row, in_=ebg)
            bg_row = const.tile([1, KC], f32, tag="bgr")
            nc.sync.dma_start(out=bg_row, in_=bgr)
            bt_row = const.tile([1, 1], f32, tag="btr")
            nc.sync.dma_start(out=bt_row, in_=bt)
            cps_row = const.tile([1, 1], f32, tag="cpsr")
            nc.sync.dma_start(out=cps_row, in_=cps)
            # partition-broadcast the tiny rows
            ebg_b = const.tile([128, KC], f32, tag="ebgb")
            nc.gpsimd.partition_broadcast(ebg_b, ebg_row)
            bg_b = const.tile([128, KC], f32, tag="bgb")
            nc.gpsimd.partition_broadcast(bg_b, bg_row)
            bt_b = const.tile([128, 1], f32, tag="btb")
            nc.gpsimd.partition_broadcast(bt_b, bt_row)
            cps_b = const.tile([128, 1], f32, tag="cpsb")
            nc.gpsimd.partition_broadcast(cps_b, cps_row)

        # ---- phase A: x^T ----
        xT = persist.tile([128, 8, N], f32r, tag="xT")
        for nt in range(NT):
            xr = xrow.tile([128, D], f32)
            nc.sync.dma_start(out=xr, in_=xb[nt * 128:(nt + 1) * 128, :])
            for ck in range(8):
                tp = psm.tile([128, 128], f32, tag="psm")
                nc.tensor.transpose(tp, xr[:, ck * 128:(ck + 1) * 128], identity)
                nc.scalar.copy(xT[:, ck, nt * 128:(nt + 1) * 128], tp)

        # ---- phase B: qkv^T  (pack ct holds heads 2ct, 2ct+1) ----
        qT = persist.tile([128, 4, N], f32r, tag="qT")
        kT = persist.tile([128, 4, N], f32r, tag="kT")
        vT = persist.tile([128, 4, N], f32, tag="vT")
        for wi, (wdram, dst) in enumerate(((wq, qT), (wk, kT), (wv, vT))):
            wre = wdram.rearrange("(c p) n -> c p n", p=128)
            wcks = []
            for ck in range(8):
                wck = wblk.tile([128, NHL * HD], f32r, tag="wblk")
                nc.sync.dma_start(out=wck, in_=wre[ck])
                wcks.append(wck)
            for ct in range(4):
                for nch in range(2):
                    ps = pa.tile([128, 512], f32, tag="pa")
                    for ck in range(8):
                        nc.tensor.matmul(
                            ps,
                            lhsT=wcks[ck][:, ct * 128:(ct + 1) * 128],
                            rhs=xT[:, ck, nch * 512:(nch + 1) * 512],
                            start=(ck == 0),
                            stop=(ck == 7),
                        )
                    d = dst[:, ct, nch * 512:(nch + 1) * 512]
                    if (ct + nch + wi) % 2 == 0:
                        nc.scalar.copy(d, ps)
                    else:
                        nc.vector.tensor_copy(d, ps)

        # ---- phase C: v natural layout + ones column (softmax sums) ----
        vnat = persist.tile([128, NHL, 8, HD + 1], bf16, tag="vnat")
        nc.vector.memset(vnat, 1.0)
        for h in range(NHL):
            o = (h % 2) * 64
            vT_h = vT[o:o + 64, h // 2, :]
            ident_h = identity[o:o + 64, o:o + 64]
            for mt in range(8):
                tp = psm.tile([128, HD], f32, tag="psm")
                nc.tensor.transpose(
                    tp, vT_h[:, mt * 128:(mt + 1) * 128], ident_h
                )
                nc.scalar.copy(vnat[:, h, mt, 0:HD], tp)

        AT = persist.tile([128, 4, N], f32r, tag="xT")

        # ---- phase D: per-head attention ----
        for h in range(NHL):
            o = (h % 2) * 64
            qT_h = qT[o:o + 64, h // 2, :]
            kT_h = kT[o:o + 64, h // 2, :]

            if with_bias:
                aff_h = aff_sb[:, h * KC:(h + 1) * KC]
                wg_h = wg_sb[o:o + HD, :]
                wt_h = wt_sb[o:o + HD, :]
                # z in natural orientation: G[128, nt, l]
                gz = psm.tile([128, NT, KC], f32, tag="psm")
                for nt in range(NT):
                    nc.tensor.matmul(
                        gz[:, nt, :],
                        lhsT=qT_h[:, nt * 128:(nt + 1) * 128].bitcast(f32),
                        rhs=wg_h,
                        start=True,
                        stop=True,
                    )
                u = smalls.tile([128, NT, KC], f32, tag="u")
                nc.scalar.activation(u, gz, AF.Exp)
                # zw = exp(G) * exp(bg); per-(token) sums over clusters
                zw = smalls.tile([128, NT, KC], f32, tag="zw")
                ebg_exp = bass.AP(
                    tensor=ebg_b.tensor,
                    offset=ebg_b.offset,
                    ap=[list(ebg_b.ap[0]), [0, NT], list(ebg_b.ap[1])],
                )
                nc.vector.tensor_tensor(out=zw, in0=u, in1=ebg_exp, op=ALU.mult)
                szt = smalls.tile([128, NT], f32, tag="szt")
                nc.vector.tensor_reduce(
                    out=szt, in_=zw, axis=mybir.AxisListType.X, op=ALU.add
                )
                nc.sync.dma_start(out=sz[h], in_=szt)
                rcp = smalls.tile([128, NT], f32, tag="rcp")
                nc.vector.reciprocal_approx_fast(rcp, szt)
                zn = smalls.tile([128, NT, KC], f32, tag="zn")
                rcp_exp = bass.AP(
                    tensor=rcp.tensor,
                    offset=rcp.offset,
                    ap=[list(rcp.ap[0]), list(rcp.ap[1]), [0, KC]],
                )
                nc.vector.tensor_tensor(out=zn, in0=zw, in1=rcp_exp, op=ALU.mult)
                # entropy pieces: zg = sum_l z * (G + bg)
                gb = smalls.tile([128, NT, KC], f32, tag="gb")
                bg_exp = bass.AP(
                    tensor=bg_b.tensor,
                    offset=bg_b.offset,
                    ap=[list(bg_b.ap[0]), [0, NT], list(bg_b.ap[1])],
                )
                nc.vector.tensor_tensor(out=gb, in0=gz, in1=bg_exp, op=ALU.add)
                zgm = smalls.tile([128, NT, KC], f32, tag="zgm")
                nc.vector.tensor_tensor(out=zgm, in0=zn, in1=gb, op=ALU.mult)
                zgt = smalls.tile([128, NT], f32, tag="zgt")
                nc.vector.tensor_reduce(
                    out=zgt, in_=zgm, axis=mybir.AxisListType.X, op=ALU.add
                )
                nc.sync.dma_start(out=zg[h], in_=zgt)
                # z^T [KC, N]
                zTn = smalls.tile([KC, N], bf16, tag="zTn")
                for nt in range(NT):
                    tpz = psm.tile([KC, 128], f32, tag="psm")
                    nc.tensor.transpose(tpz, zn[:, nt, :], identity)
                    nc.scalar.copy(zTn[:, nt * 128:(nt + 1) * 128], tpz)
                # Y = (z B)^T [KC, N]
                Y = smalls.tile([KC, N], bf16, tag="Y")
                for nch in range(2):
                    py = psm.tile([KC, 512], f32, tag="psm")
                    nc.tensor.matmul(
                        py,
                        lhsT=aff_h,
                        rhs=zTn[:, nch * 512:(nch + 1) * 512],
                        start=True,
                        stop=True,
                    )
                    nc.scalar.copy(Y[:, nch * 512:(nch + 1) * 512], py)
                # theta row [1, N] (for the free-dim factor, broadcast below)
                throw = smalls.tile([1, N], bf16, tag="throw")
                for nch in range(2):
                    pth = psm.tile([1, 512], f32, tag="psm")
                    nc.tensor.matmul(
                        pth,
                        lhsT=wt_h,
                        rhs=qT_h[:, nch * 512:(nch + 1) * 512].bitcast(f32),
                        start=True,
                        stop=True,
                    )
                    nc.scalar.activation(
                        throw[:, nch * 512:(nch + 1) * 512], pth, AF.Relu, bias=bt_row
                    )
                thn = bcast.tile([128, N], bf16, tag="thn")
                nc.gpsimd.partition_broadcast(thn, throw)
                # theta natural [128, nt] (per-partition factor), scaled by cps
                pthn = psm.tile([128, NT], f32, tag="psm")
                for nt in range(NT):
                    nc.tensor.matmul(
                        pthn[:, nt:nt + 1],
                        lhsT=qT_h[:, nt * 128:(nt + 1) * 128].bitcast(f32),
                        rhs=wt_h,
                        start=True,
                        stop=True,
                    )
                thsc = smalls.tile([128, NT], f32, tag="thsc")
                nc.scalar.activation(thsc, pthn, AF.Relu, bias=bt_b)
                nc.vector.tensor_scalar_mul(thsc, thsc, cps_b)

            # attention over key tiles
            pav0 = pav.tile([HD + 1, 512], f32, tag="pav")
            pav1 = pav.tile([HD + 1, 512], f32, tag="pav")
            for mt in range(8):
                pt_tile = ptile.tile([128, N], bf16, tag="pt")
                for nch in range(2):
                    ps_s = pa.tile([128, 512], f32, tag="pa")
                    nc.tensor.matmul(
                        ps_s,
                        lhsT=kT_h[:, mt * 128:(mt + 1) * 128],
                        rhs=qT_h[:, nch * 512:(nch + 1) * 512],
                        start=True,
                        stop=True,
                    )
                    if with_bias:
                        ps_c = pa.tile([128, 512], f32, tag="pa")
                        nc.tensor.matmul(
                            ps_c,
                            lhsT=zTn[:, mt * 128:(mt + 1) * 128],
                            rhs=Y[:, nch * 512:(nch + 1) * 512],
                            start=True,
                            stop=True,
                        )
                        tt = work.tile([128, 512], bf16, tag="T")
                        nc.scalar.activation(tt, ps_c, AF.Tanh)
                        cp = cpbp.tile([128, 512], f32, tag="cp")
                        nc.vector.scalar_tensor_tensor(
                            out=cp,
                            in0=tt,
                            scalar=thsc[:, mt:mt + 1],
                            in1=thn[:, nch * 512:(nch + 1) * 512],
                            op0=ALU.mult,
                            op1=ALU.mult,
                        )
                        nc.sync.dma_start(
                            out=cpbT[
                                h,
                                mt * 128:(mt + 1) * 128,
                                nch * 512:(nch + 1) * 512,
                            ],
                            in_=cp,
                        )
                        ll = work.tile([128, 512], bf16, tag="L")
                        nc.vector.tensor_tensor(out=ll, in0=ps_s, in1=cp, op=ALU.add)
                        nc.scalar.activation(
                            pt_tile[:, nch * 512:(nch + 1) * 512], ll, AF.Exp
                        )
                    else:
                        nc.scalar.activation(
                            pt_tile[:, nch * 512:(nch + 1) * 512], ps_s, AF.Exp
                        )
                for nch in range(2):
                    nc.tensor.matmul(
                        pav0 if nch == 0 else pav1,
                        lhsT=vnat[:, h, mt, :],
                        rhs=pt_tile[:, nch * 512:(nch + 1) * 512],
                        start=(mt == 0),
                        stop=(mt == 7),
                    )

            # normalize and store A^T into the projection pack
            s_sb = tail.tile([1, N], f32, tag="s")
            nc.vector.tensor_copy(s_sb[:, 0:512], pav0[64:65, :])
            nc.vector.tensor_copy(s_sb[:, 512:N], pav1[64:65, :])
            nc.vector.reciprocal_approx_fast(s_sb, s_sb)
            rcp_b = bcast.tile([128, N], f32, tag="rb")
            nc.gpsimd.partition_broadcast(rcp_b, s_sb)
            at_dst = AT[o:o + 64, h // 2, :]
            nc.scalar.copy(at_dst[:, 0:512], pav0[0:64, :])
            nc.scalar.copy(at_dst[:, 512:N], pav1[0:64, :])
            nc.vector.tensor_tensor(
                out=at_dst, in0=at_dst, in1=rcp_b[o:o + 64, :], op=ALU.mult
            )

        # ---- projection ----
        wpsb = persist.tile([128, 4, D], f32r, tag="wp")
        nc.sync.dma_start(out=wpsb, in_=wp.rearrange("(c p) e -> p c e", p=128))
        for nt in range(NT):
            for ech in range(2):
                po = pa.tile([128, 512], f32, tag="pa")
                for pt_ in range(4):
                    nc.tensor.matmul(
                        po,
                        lhsT=AT[:, pt_, nt * 128:(nt + 1) * 128],
                        rhs=wpsb[:, pt_, ech * 512:(ech + 1) * 512],
                        start=(pt_ == 0),
                        stop=(pt_ == 3),
                    )
                ot = opool.tile([128, 512], f32, tag="ot")
                if ech == 0:
                    nc.scalar.copy(ot, po)
                else:
                    nc.vector.tensor_copy(ot, po)
                nc.sync.dma_start(
                    out=o_part[nt * 128:(nt + 1) * 128, ech * 512:(ech + 1) * 512],
                    in_=ot,
                )

    nc.finalize()
    return nc


def _get_graph(with_bias: bool):
    if with_bias not in _graph_cache:
        _graph_cache[with_bias] = _build(with_bias)
    return _graph_cache[with_bias]


def kernel(x, Wqkv, Wproj, bproj, Wgroup, bgroup, Wtheta, btheta,
           affinity_B, cp_bias_scale, flag, injection, cp_dcmm):
    from concourse.bass_utils import run_bass_kernel_spmd

    x = np.ascontiguousarray(np.asarray(x, np.float32))
    Wqkv = np.ascontiguousarray(np.asarray(Wqkv, np.float32))
    Wproj = np.ascontiguousarray(np.asarray(Wproj, np.float32))
    bproj = np.asarray(bproj, np.float32)
    with_bias = bool(int(np.asarray(cp_dcmm)) and int(np.asarray(flag)))

    nc = _get_graph(with_bias)
    in_maps = []
    for c in range(8):
        b = c // 2
        hg = (c % 2) * NHL
        m = {
            "xb": x[b],
            "wq": np.ascontiguousarray(Wqkv[:, hg * HD:hg * HD + NHL * HD]),
            "wk": np.ascontiguousarray(
                Wqkv[:, D + hg * HD:D + hg * HD + NHL * HD]
            ) * np.float32(SCALE),
            "wv": np.ascontiguousarray(
                Wqkv[:, 2 * D + hg * HD:2 * D + hg * HD + NHL * HD]
            ),
            "wp": np.ascontiguousarray(Wproj[hg * HD:hg * HD + NHL * HD, :]),
        }
        if with_bias:
            Wgroup = np.asarray(Wgroup, np.float32)
            bgroup = np.asarray(bgroup, np.float32)
            Wtheta = np.asarray(Wtheta, np.float32)
            btheta = np.asarray(btheta, np.float32)
            affs = 1.0 / (1.0 + np.exp(-np.asarray(affinity_B, np.float32)))
            m |= {
                "wg": Wgroup,
                "ebg": np.exp(bgroup)[None, :].astype(np.float32),
                "bgr": bgroup[None, :],
                "wt": Wtheta,
                "bt": btheta[None, :],
                # [k, (h_local, l)]
                "aff": np.ascontiguousarray(
                    affs[hg:hg + NHL].transpose(1, 0, 2).reshape(KC, NHL * KC)
                ),
                "cps": np.asarray(cp_bias_scale, np.float32).reshape(1, 1),
            }
        in_maps.append(m)

    res = run_bass_kernel_spmd(nc, in_maps, core_ids=list(range(8))).results

    B = x.shape[0]
    out = np.zeros([B, N, D], np.float32)
    for c in range(8):
        out[c // 2] += res[c]["o_part"]
    out += bproj[None, None, :]

    if with_bias:
        cpb = np.empty([B, 2 * NHL, N, N], np.float32)
        tot = 0.0
        for c in range(8):
            hg = (c % 2) * NHL
            cpb[c // 2, hg:hg + NHL] = res[c]["cpbT"].transpose(0, 2, 1)
            tot += np.log(res[c]["sz"].astype(np.float64)).sum()
            tot -= res[c]["zg"].astype(np.float64).sum()
        zc = np.float32(tot / (B * 2 * NHL * N))
    else:
        cpb = np.zeros([B, 2 * NHL, N, N], np.float32)
        zc = np.float32(0.0)

    zero = np.float32(0.0)
    return (out, zc, zero, zero, cpb)


# revision 18
# speedup vs baseline: 1.2779x; 1.2779x over previous
"""Trainium2 Bass kernel for nn_Attention_65601330479503.

Cluster-pattern-biased multi-head attention:
  qkv = x @ Wqkv; attn = softmax(q k^T/sqrt(hd) + cp_bias); out = (attn v) @ Wproj
  cp_bias = cp_scale * theta_i theta_j * tanh(zB_i . z_j),  z = softmax(q Wgroup)

Sharding: 8 cores, core c -> batch b = c//2, head-group hg = (c%2)*8 (8 of 16
heads).  Each core computes its 8 heads' attention fully on-device; the host
adds the two per-batch partial output projections and transposes the per-core
cp_bias^T slabs into the full [B, H, N, N] output.

On-device orientation is TRANSPOSED ([key m on partitions, query n on free])
so that softmax sums come free via an appended ones-column in v (row 64 of the
AV accumulator) and attn@v needs no P-matrix transposes.  cp_bias is emitted
as [m, n] tiles and un-transposed on the host during the gather.

All PE-facing tensors are bf16 (f32 weight loads are slow and FWL-ineligible);
PSUM accumulation stays f32.  The z/theta projections share one fused 9-column
matmul per n-chunk in transposed orientation; all per-head z work (phase D0)
is hoisted ahead of the attention loops (D1) to keep the PE HAM-warm.
cp_bias is computed bf16 and cast to f32 by the store DMA (SWDGE).
"""

import sys

import numpy as np

sys.path.insert(0, "/opt/trn_rl_repo")

N = 1024          # sequence length
D = 1024          # model dim
HD = 64           # head dim
NHL = 8           # heads per core (local)
KC = 8            # clusters
NT = N // 128     # 8 n-tiles
SCALE = HD ** -0.5

_graph_cache = {}


def _build(with_bias: bool):
    from contextlib import ExitStack

    import concourse.bass as bass
    import concourse.tile as tile
    from concourse import bacc, mybir
    from concourse.masks import make_identity

    f32 = mybir.dt.float32
    bf16 = mybir.dt.bfloat16
    AF = mybir.ActivationFunctionType
    ALU = mybir.AluOpType

    nc = bacc.Bacc("TRN2", target_bir_lowering=False, debug=False, num_devices=8)

    xb = nc.dram_tensor("xb", [N, D], bf16, kind="ExternalInput").ap()
    wq = nc.dram_tensor("wq", [D, NHL * HD], bf16, kind="ExternalInput").ap()
    wk = nc.dram_tensor("wk", [D, NHL * HD], bf16, kind="ExternalInput").ap()
    wv = nc.dram_tensor("wv", [D, NHL * HD], bf16, kind="ExternalInput").ap()
    wp = nc.dram_tensor("wp", [NHL * HD, D], bf16, kind="ExternalInput").ap()
    o_part = nc.dram_tensor("o_part", [N, D], f32, kind="ExternalOutput").ap()
    if with_bias:
        # fused [Wgroup | Wtheta] (9 columns)
        wgt = nc.dram_tensor("wgt", [HD, KC + 1], bf16, kind="ExternalInput").ap()
        bgr = nc.dram_tensor("bgr", [KC, 1], f32, kind="ExternalInput").ap()
        bt = nc.dram_tensor("bt", [1, 1], f32, kind="ExternalInput").ap()
        aff = nc.dram_tensor("aff", [KC, NHL * KC], bf16, kind="ExternalInput").ap()
        cps = nc.dram_tensor("cps", [1, 1], f32, kind="ExternalInput").ap()
        cpbT = nc.dram_tensor("cpbT", [NHL, N, N], f32, kind="ExternalOutput").ap()
        sz = nc.dram_tensor("sz", [NHL, N], f32, kind="ExternalOutput").ap()
        zg = nc.dram_tensor("zg", [NHL, N], f32, kind="ExternalOutput").ap()
        scrth = nc.dram_tensor("scrth", [NHL, N], bf16).ap()

    with tile.TileContext(nc) as tc, ExitStack() as ctx:
        const = ctx.enter_context(tc.tile_pool(name="const", bufs=1))
        persist = ctx.enter_context(tc.tile_pool(name="persist", bufs=1))
        xrow = ctx.enter_context(tc.tile_pool(name="xrow", bufs=2))
        wblk = ctx.enter_context(tc.tile_pool(name="wblk", bufs=8))
        ptile = ctx.enter_context(tc.tile_pool(name="ptile", bufs=2))
        work = ctx.enter_context(tc.tile_pool(name="work", bufs=3))
        cpbp = ctx.enter_context(tc.tile_pool(name="cpbp", bufs=3))
        bcast = ctx.enter_context(tc.tile_pool(name="bcast", bufs=2))
        zpool = ctx.enter_context(tc.tile_pool(name="zpool", bufs=NHL))
        smalls = ctx.enter_context(tc.tile_pool(name="smalls", bufs=1))
        tail = ctx.enter_context(tc.tile_pool(name="tail", bufs=1))
        opool = ctx.enter_context(tc.tile_pool(name="opool", bufs=2))
        paS = ctx.enter_context(tc.tile_pool(name="paS", bufs=2, space="PSUM"))
        paC = ctx.enter_context(tc.tile_pool(name="paC", bufs=2, space="PSUM"))
        pavm = ctx.enter_context(tc.tile_pool(name="pavm", bufs=2, space="PSUM"))

        identity = const.tile([128, 128], bf16)
        make_identity(nc, identity)

        if with_bias:
            wgt_sb = const.tile([128, KC + 1], bf16, tag="wgt")
            nc.sync.dma_start(out=wgt_sb[0:HD, :], in_=wgt)
            nc.sync.dma_start(out=wgt_sb[HD:2 * HD, :], in_=wgt)
            aff_sb = const.tile([KC, NHL * KC], bf16, tag="aff")
            nc.sync.dma_start(out=aff_sb, in_=aff)
            bg_sb = const.tile([KC, 1], f32, tag="bgs")
            nc.sync.dma_start(out=bg_sb, in_=bgr)
            bt_row = const.tile([1, 1], f32, tag="btr")
            nc.sync.dma_start(out=bt_row, in_=bt)
            cps_row = const.tile([1, 1], f32, tag="cpsr")
            nc.sync.dma_start(out=cps_row, in_=cps)
            bt_b = const.tile([128, 1], f32, tag="btb")
            nc.gpsimd.partition_broadcast(bt_b, bt_row)
            cps_b = const.tile([128, 1], f32, tag="cpsb")
            nc.gpsimd.partition_broadcast(cps_b, cps_row)
            ones8 = const.tile([KC, 1], bf16, tag="ones8")
            nc.gpsimd.memset(ones8, 1.0)

        # ---- phase A: x^T (bf16), copies batched 4 transposes per bank ----
        xT = persist.tile([128, 8, N], bf16, tag="xT")
        for nt in range(NT):
            xr = xrow.tile([128, D], bf16)
            nc.sync.dma_start(out=xr, in_=xb[nt * 128:(nt + 1) * 128, :])
            for g in range(2):
                bank = pavm.tile([128, 4, 128], bf16, tag="pavm")
                for i in range(4):
                    ck = g * 4 + i
                    nc.tensor.transpose(
                        bank[:, i, :], xr[:, ck * 128:(ck + 1) * 128], identity
                    )
                nc.scalar.copy(
                    xT[:, g * 4:(g + 1) * 4, nt * 128:(nt + 1) * 128], bank
                )

        # ---- phase B: qkv^T (pack ct holds heads 2ct, 2ct+1) ----
        qT = persist.tile([128, 4, N], bf16, tag="qT")
        kT = persist.tile([128, 4, N], bf16, tag="kT")
        vT = persist.tile([128, 4, N], bf16, tag="vT")
        for wi, (wdram, dst) in enumerate(((wq, qT), (wk, kT), (wv, vT))):
            wre = wdram.rearrange("(c p) n -> c p n", p=128)
            wcks = []
            for ck in range(8):
                wck = wblk.tile([128, NHL * HD], bf16, tag="wblk")
                nc.sync.dma_start(out=wck, in_=wre[ck])
                wcks.append(wck)
            for ct in range(4):
                for nch in range(2):
                    ps = paS.tile([128, 512], f32, tag="paS")
                    for ck in range(8):
                        nc.tensor.matmul(
                            ps,
                            lhsT=wcks[ck][:, ct * 128:(ct + 1) * 128],
                            rhs=xT[:, ck, nch * 512:(nch + 1) * 512],
                            start=(ck == 0),
                            stop=(ck == 7),
                        )
                    d = dst[:, ct, nch * 512:(nch + 1) * 512]
                    if (ct + nch + wi) % 2 == 0:
                        nc.scalar.copy(d, ps)
                    else:
                        nc.vector.tensor_copy(d, ps)

        # ---- phase C: v natural layout + ones column; one copy per head ----
        vnat = persist.tile([128, NHL, 8, HD + 1], bf16, tag="vnat")
        nc.gpsimd.memset(vnat, 1.0)
        for h in range(NHL):
            o = (h % 2) * 64
            vT_h = vT[o:o + 64, h // 2, :]
            ident_h = identity[o:o + 64, o:o + 64]
            bank = pavm.tile([128, 8, HD], bf16, tag="pavm")
            for mt in range(8):
                nc.tensor.transpose(
                    bank[:, mt, :], vT_h[:, mt * 128:(mt + 1) * 128], ident_h
                )
            nc.scalar.copy(vnat[:, h, :, 0:HD], bank)

        AT = persist.tile([128, 4, N], bf16, tag="AT")

        # ---- phase D0: z / theta for every head (transposed orientation) ----
        if with_bias:
            zTns, Ys, thns, thscs = [], [], [], []
            for h in range(NHL):
                o = (h % 2) * 64
                qT_h = qT[o:o + 64, h // 2, :]
                wgt_h = wgt_sb[o:o + HD, :]
                aff_h = aff_sb[:, h * KC:(h + 1) * KC]

                # G^T [8, N] in one 2-bank psum tile
                gps = paC.tile([KC, N], f32, tag="paC")
                for nch in range(2):
                    nc.tensor.matmul(
                        gps[:, nch * 512:(nch + 1) * 512],
                        lhsT=wgt_h[:, 0:KC],
                        rhs=qT_h[:, nch * 512:(nch + 1) * 512],
                        start=True,
                        stop=True,
                    )
                # theta row = relu(q Wtheta + bt)  [1, N] bf16
                throw = smalls.tile([1, N], bf16, tag="throw")
                for nch in range(2):
                    tps = pavm.tile([1, 512], f32, tag="pavm", name=f"tps{h}_{nch}")
                    nc.tensor.matmul(
                        tps,
                        lhsT=wgt_h[:, KC:KC + 1],
                        rhs=qT_h[:, nch * 512:(nch + 1) * 512],
                        start=True,
                        stop=True,
                    )
                    nc.scalar.activation(
                        throw[:, nch * 512:(nch + 1) * 512], tps, AF.Relu,
                        bias=bt_row,
                    )
                # u = exp(G + bg)  [8, N] bf16
                expg = smalls.tile([KC, N], bf16, tag="expg")
                nc.scalar.activation(expg, gps, AF.Exp, bias=bg_sb)
                # G' = G + bg (bf16, for the entropy term)
                gpr = smalls.tile([KC, N], bf16, tag="gpr")
                nc.scalar.activation(gpr, gps, AF.Identity, bias=bg_sb)
                # sz row: column sums of u via ones-matmul
                szp = paC.tile([1, N], f32, tag="paC")
                for nch in range(2):
                    nc.tensor.matmul(
                        szp[:, nch * 512:(nch + 1) * 512],
                        lhsT=ones8,
                        rhs=expg[:, nch * 512:(nch + 1) * 512],
                        start=True,
                        stop=True,
                    )
                sz_row = smalls.tile([1, N], f32, tag="szrow")
                nc.vector.tensor_copy(sz_row, szp)
                nc.sync.dma_start(out=sz[h], in_=sz_row)
                rcp_row = smalls.tile([1, N], f32, tag="rcprow")
                nc.vector.reciprocal_approx_fast(rcp_row, sz_row)
                rcp_b8 = smalls.tile([KC, N], f32, tag="rcpb8")
                nc.gpsimd.partition_broadcast(rcp_b8, rcp_row)
                # z^T normalized (bf16)
                zTn = zpool.tile([KC, N], bf16, tag="zTn")
                nc.vector.tensor_tensor(out=zTn, in0=expg, in1=rcp_b8, op=ALU.mult)
                # entropy partial: zg row = colsum(z * G')
                zG = smalls.tile([KC, N], bf16, tag="zG")
                nc.vector.tensor_tensor(out=zG, in0=zTn, in1=gpr, op=ALU.mult)
                zgp = paC.tile([1, N], f32, tag="paC")
                for nch in range(2):
                    nc.tensor.matmul(
                        zgp[:, nch * 512:(nch + 1) * 512],
                        lhsT=ones8,
                        rhs=zG[:, nch * 512:(nch + 1) * 512],
                        start=True,
                        stop=True,
                    )
                zg_row = smalls.tile([1, N], f32, tag="zgrow")
                nc.vector.tensor_copy(zg_row, zgp)
                nc.sync.dma_start(out=zg[h], in_=zg_row)
                # Y = (z B)^T [8, N] bf16
                Y = zpool.tile([KC, N], bf16, tag="Y")
                for nch in range(2):
                    yp = pavm.tile([KC, 512], f32, tag="pavm")
                    nc.tensor.matmul(
                        yp,
                        lhsT=aff_h,
                        rhs=zTn[:, nch * 512:(nch + 1) * 512],
                        start=True,
                        stop=True,
                    )
                    nc.scalar.copy(Y[:, nch * 512:(nch + 1) * 512], yp)
                # theta broadcast over free dim
                thn = zpool.tile([128, N], bf16, tag="thn")
                nc.gpsimd.partition_broadcast(thn, throw)
                # theta natural [128, NT] via DRAM bounce, scaled by cps
                nc.sync.dma_start(out=scrth[h], in_=throw)
                thp = smalls.tile([128, NT], bf16, tag="thp")
                nc.sync.dma_start(
                    out=thp, in_=scrth[h].rearrange("(t p) -> p t", p=128)
                )
                thsc = zpool.tile([128, NT], f32, tag="thsc")
                nc.vector.tensor_scalar_mul(thsc, thp, cps_b)

                zTns.append(zTn)
                Ys.append(Y)
                thns.append(thn)
                thscs.append(thsc)

        # ---- phase D1: attention (software-pipelined AV) ----
        for h in range(NHL):
            o = (h % 2) * 64
            qT_h = qT[o:o + 64, h // 2, :]
            kT_h = kT[o:o + 64, h // 2, :]
            if with_bias:
                zTn, Y, thn, thsc = zTns[h], Ys[h], thns[h], thscs[h]

            pav0 = pavm.tile([HD + 1, 512], f32, tag="pavm")
            pav1 = pavm.tile([HD + 1, 512], f32, tag="pavm")
            pts = [None] * 8
            for mt in range(8):
                pt_tile = ptile.tile([128, N], bf16, tag="pt")
                pts[mt] = pt_tile
                ps_s = [None, None]
                for nch in range(2):
                    ps_s[nch] = paS.tile(
                        [128, 512], f32, tag="paS", name=f"psS{h}_{mt}_{nch}"
                    )
                    nc.tensor.matmul(
                        ps_s[nch],
                        lhsT=kT_h[:, mt * 128:(mt + 1) * 128],
                        rhs=qT_h[:, nch * 512:(nch + 1) * 512],
                        start=True,
                        stop=True,
                    )
                if with_bias:
                    ps_c = paC.tile([128, N], f32, tag="paC")
                    for nch in range(2):
                        nc.tensor.matmul(
                            ps_c[:, nch * 512:(nch + 1) * 512],
                            lhsT=zTn[:, mt * 128:(mt + 1) * 128],
                            rhs=Y[:, nch * 512:(nch + 1) * 512],
                            start=True,
                            stop=True,
                        )
                    tt = work.tile([128, N], bf16, tag="T")
                    nc.scalar.activation(tt, ps_c, AF.Tanh)
                    cp = cpbp.tile([128, N], bf16, tag="cp")
                    nc.vector.scalar_tensor_tensor(
                        out=cp,
                        in0=tt,
                        scalar=thsc[:, mt:mt + 1],
                        in1=thn,
                        op0=ALU.mult,
                        op1=ALU.mult,
                    )
                    nc.gpsimd.dma_start(
                        out=cpbT[h, mt * 128:(mt + 1) * 128, :], in_=cp
                    )
                    ll = work.tile([128, N], bf16, tag="L")
                    for nch in range(2):
                        nc.vector.tensor_tensor(
                            out=ll[:, nch * 512:(nch + 1) * 512],
                            in0=ps_s[nch],
                            in1=cp[:, nch * 512:(nch + 1) * 512],
                            op=ALU.add,
                        )
                    nc.scalar.activation(pt_tile, ll, AF.Exp)
                else:
                    for nch in range(2):
                        nc.scalar.activation(
                            pt_tile[:, nch * 512:(nch + 1) * 512],
                            ps_s[nch],
                            AF.Exp,
                        )
                if mt > 0:
                    for nch, pav in ((0, pav0), (1, pav1)):
                        nc.tensor.matmul(
                            pav,
                            lhsT=vnat[:, h, mt - 1, :],
                            rhs=pts[mt - 1][:, nch * 512:(nch + 1) * 512],
                            start=(mt - 1 == 0),
                            stop=False,
                        )
            for nch, pav in ((0, pav0), (1, pav1)):
                nc.tensor.matmul(
                    pav,
                    lhsT=vnat[:, h, 7, :],
                    rhs=pts[7][:, nch * 512:(nch + 1) * 512],
                    start=False,
                    stop=True,
                )

            # normalize and store A^T into the projection pack
            s_sb = tail.tile([1, N], f32, tag="s")
            nc.vector.tensor_copy(s_sb[:, 0:512], pav0[64:65, :])
            nc.vector.tensor_copy(s_sb[:, 512:N], pav1[64:65, :])
            nc.vector.reciprocal_approx_fast(s_sb, s_sb)
            rcp_bf = tail.tile([1, N], bf16, tag="rbf")
            nc.vector.tensor_copy(rcp_bf, s_sb)
            rcp_b = bcast.tile([128, N], bf16, tag="rb")
            nc.gpsimd.partition_broadcast(rcp_b, rcp_bf)
            at_dst = AT[o:o + 64, h // 2, :]
            nc.scalar.copy(at_dst[:, 0:512], pav0[0:64, :])
            nc.scalar.copy(at_dst[:, 512:N], pav1[0:64, :])
            nc.vector.tensor_tensor(
                out=at_dst, in0=at_dst, in1=rcp_b[o:o + 64, :], op=ALU.mult
            )

        # ---- projection ----
        wpsb = persist.tile([128, 4, D], bf16, tag="wp")
        nc.sync.dma_start(out=wpsb, in_=wp.rearrange("(c p) e -> p c e", p=128))
        for nt in range(NT):
            for ech in range(2):
                po = paS.tile([128, 512], f32, tag="paS")
                for pt_ in range(4):
                    nc.tensor.matmul(
                        po,
                        lhsT=AT[:, pt_, nt * 128:(nt + 1) * 128],
                        rhs=wpsb[:, pt_, ech * 512:(ech + 1) * 512],
                        start=(pt_ == 0),
                        stop=(pt_ == 3),
                    )
                ot = opool.tile([128, 512], f32, tag="ot")
                if ech == 0:
                    nc.scalar.copy(ot, po)
                else:
                    nc.vector.tensor_copy(ot, po)
                nc.sync.dma_start(
                    out=o_part[nt * 128:(nt + 1) * 128, ech * 512:(ech + 1) * 512],
                    in_=ot,
                )

    nc.finalize()
    return nc


def _get_graph(with_bias: bool):
    if with_bias not in _graph_cache:
        _graph_cache[with_bias] = _build(with_bias)
    return _graph_cache[with_bias]


def kernel(x, Wqkv, Wproj, bproj, Wgroup, bgroup, Wtheta, btheta,
           affinity_B, cp_bias_scale, flag, injection, cp_dcmm):
    import ml_dtypes

    from concourse.bass_utils import run_bass_kernel_spmd

    bf = ml_dtypes.bfloat16
    x = np.asarray(x, np.float32)
    Wqkv = np.asarray(Wqkv, np.float32)
    Wproj = np.asarray(Wproj, np.float32)
    bproj = np.asarray(bproj, np.float32)
    with_bias = bool(int(np.asarray(cp_dcmm)) and int(np.asarray(flag)))

    nc = _get_graph(with_bias)
    in_maps = []
    for c in range(8):
        b = c // 2
        hg = (c % 2) * NHL
        m = {
            "xb": np.ascontiguousarray(x[b]).astype(bf),
            "wq": np.ascontiguousarray(
                Wqkv[:, hg * HD:hg * HD + NHL * HD]).astype(bf),
            "wk": (np.ascontiguousarray(
                Wqkv[:, D + hg * HD:D + hg * HD + NHL * HD])
                * np.float32(SCALE)).astype(bf),
            "wv": np.ascontiguousarray(
                Wqkv[:, 2 * D + hg * HD:2 * D + hg * HD + NHL * HD]).astype(bf),
            "wp": np.ascontiguousarray(
                Wproj[hg * HD:hg * HD + NHL * HD, :]).astype(bf),
        }
        if with_bias:
            Wg = np.asarray(Wgroup, np.float32)
            bg = np.asarray(bgroup, np.float32)
            Wt = np.asarray(Wtheta, np.float32)
            btv = np.asarray(btheta, np.float32)
            affs = 1.0 / (1.0 + np.exp(-np.asarray(affinity_B, np.float32)))
            m |= {
                "wgt": np.concatenate([Wg, Wt], axis=1).astype(bf),
                "bgr": np.ascontiguousarray(bg[:, None]),
                "bt": btv.reshape(1, 1),
                # [k, (h_local, l)]
                "aff": np.ascontiguousarray(
                    affs[hg:hg + NHL].transpose(1, 0, 2).reshape(KC, NHL * KC)
                ).astype(bf),
                "cps": np.asarray(cp_bias_scale, np.float32).reshape(1, 1),
            }
        in_maps.append(m)

    res = run_bass_kernel_spmd(nc, in_maps, core_ids=list(range(8))).results

    B = x.shape[0]
    out = np.zeros([B, N, D], np.float32)
    for c in range(8):
        out[c // 2] += res[c]["o_part"]
    out += bproj[None, None, :]

    if with_bias:
        cpb = np.empty([B, 2 * NHL, N, N], np.float32)
        tot = 0.0
        for c in range(8):
            hg = (c % 2) * NHL
            cpb[c // 2, hg:hg + NHL] = res[c]["cpbT"].transpose(0, 2, 1)
            tot += np.log(res[c]["sz"].astype(np.float64)).sum()
            tot -= res[c]["zg"].astype(np.float64).sum()
        zc = np.float32(tot / (B * 2 * NHL * N))
    else:
        cpb = np.zeros([B, 2 * NHL, N, N], np.float32)
        zc = np.float32(0.0)

    zero = np.float32(0.0)
    return (out, zc, zero, zero, cpb)
